# revision 1
# baseline (speedup 1.0000x reference)
"""Causal self-attention (B=4, S=2048, D=1024, H=16) on 8 TRN2 NeuronCores.

Sharding (tensor-parallel on heads + data-parallel on batch):
  core c -> batch c//2, head-half c%2 (8 of 16 heads).
  Wq/Wk/Wv column-split, Wo row-split; the two partial outputs per batch are
  summed on the host (+ bo), which is the row-parallel unshard.

Per-core Bass/Tile program (matmul operands bf16, psum/softmax fp32):
  phase A: qT/kT feature-major projections (4-moving-block stationary chains);
           v token-major with a per-head ones column, emitted per-superblock
           interleaved with attention to keep the PE stream dense.
  phase B: per head / 512-query superblock / 128-key tile:
           scoresT = k_j @ q_blk.T (keys on partitions, two heads on disjoint
           PE row groups), additive triangular mask on the diagonal boundary
           subtile, dead columns skipped in scores, exp and PV
           (no max subtraction: scores ~ N(0,1)); PV accumulation with the
           ones column producing sumexp in row 64; reciprocal broadcast via a
           K=1 matmul; PV emission software-pipelined one key tile behind
           scores to hide the exp latency.
  phase C: out_partial = attnT.T @ Wo_rows (stationary reused across the two
           output column blocks).
"""

from contextlib import ExitStack

import numpy as np
import ml_dtypes

import concourse.bass as bass
import concourse.bacc as bacc
import concourse.tile as tile
import concourse.mybir as mybir

F32 = mybir.dt.float32
F32R = mybir.dt.float32r
BF16 = mybir.dt.bfloat16
NEG = -30000.0  # additive mask; must stay finite-representable in bf16 paths


def r(ap):
    return ap.bitcast(F32R)


def build_core_program(S=2048, D=1024, HC=8, DH=64, SQ=512, mm_dt=BF16,
                       xt_bufs=2, qk_psum_bufs=4, probs_bufs=6):
    """Build the per-core Bass program (SPMD: same program, different data).
    mm_dt: dtype of matmul operands (BF16 or F32R). When BF16, the host must
    pass xT/wqk/wv/wo as bfloat16 arrays."""
    DQ = HC * DH              # head-slice width (512)
    DK = D // 128             # contraction tiles for projections (8)
    DQN = DQ // 128           # head-pair tiles (4)
    NSB = S // SQ             # query superblocks (4)
    NTT = S // 128            # token tiles (16)
    NOUT = min(512, D)        # output-proj free width
    NOB = D // NOUT           # output-proj col blocks (2)
    assert DQ % 128 == 0 and S % SQ == 0 and SQ % 128 == 0 and D % 128 == 0
    assert (S // SQ) % 2 == 0

    bf = mm_dt == BF16
    in_dt = BF16 if bf else F32

    def m(ap):
        # bitcast for f32->f32r reinterpretation; no-op for bf16 tiles
        return ap if bf else ap.bitcast(F32R)

    nc = bacc.Bacc("TRN2", target_bir_lowering=False, debug=False)

    xT = nc.dram_tensor("xT", [D, S], in_dt, kind="ExternalInput").ap()
    wqk = nc.dram_tensor("wqk", [D, 2 * DQ], in_dt, kind="ExternalInput").ap()
    wv = nc.dram_tensor("wv", [D, DQ], in_dt, kind="ExternalInput").ap()
    wo = nc.dram_tensor("wo", [DQ, D], in_dt, kind="ExternalInput").ap()
    bqk = nc.dram_tensor("bqk", [2 * DQ], F32, kind="ExternalInput").ap()
    bv = nc.dram_tensor("bv", [DQ], F32, kind="ExternalInput").ap()
    out = nc.dram_tensor("out", [S, D], F32, kind="ExternalOutput").ap()

    with tile.TileContext(nc) as tc, ExitStack() as ctx:
        ctx.enter_context(nc.allow_low_precision(
            reason="low-precision matmul operands; accumulation stays fp32"))
        const = ctx.enter_context(tc.tile_pool(name="const", bufs=1))
        big = ctx.enter_context(tc.tile_pool(name="big", bufs=1))
        stream = ctx.enter_context(tc.tile_pool(name="stream", bufs=1))
        psum = ctx.enter_context(tc.tile_pool(name="psum", bufs=1, space="PSUM"))

        # ---- constants ----
        # triangular mask [128,128]: 0 where p <= f else NEG (boundary subtile)
        tri = const.tile([128, 128], F32)
        nc.vector.memset(tri[:], 0.0)
        nc.gpsimd.affine_select(
            out=tri[:], in_=tri[:], compare_op=mybir.AluOpType.is_ge,
            fill=NEG, base=0, channel_multiplier=-1, pattern=[[1, 128]],
        )
        ones128f = const.tile([1, 128], F32)
        nc.vector.memset(ones128f[:], 1.0)
        ones64r = const.tile([1, 64], F32R)
        nc.vector.tensor_copy(ones64r[:], ones128f[:, 0:64])
        ones128r = const.tile([1, 128], F32R)
        nc.vector.tensor_copy(ones128r[:], ones128f[:])
        ones_hc = const.tile([128, HC], F32)
        nc.vector.memset(ones_hc[:], 1.0)

        # biases: bqk as [128, 2*DQN] (column t = dout tile t), bv broadcast
        bqk_sb = const.tile([128, 2 * DQN], F32)
        nc.sync.dma_start(bqk_sb[:], bqk.rearrange("(t p) -> p t", p=128))
        bv_rowf = const.tile([1, DQ], F32)
        nc.sync.dma_start(bv_rowf[:], bv.rearrange("(a d) -> a d", a=1))
        bv_row = const.tile([1, DQ], F32R)
        nc.vector.tensor_copy(bv_row[:], bv_rowf[:])
        bv_bc = const.tile([128, DQ], F32)
        bv_ps = psum.tile([128, DQ], F32, tag="v", bufs=2)
        nc.tensor.matmul(bv_ps[:], r(ones128r[:]), r(bv_row[:]),
                         start=True, stop=True)
        nc.scalar.copy(bv_bc[:], bv_ps[:])

        # ---- big resident tensors ----
        kT = big.tile([128, DQN, S], mm_dt)     # [pair 2x64 rows, tokens]
        qT = big.tile([128, DQN, S], mm_dt)
        v_aug = big.tile([128, NTT, HC * 65], mm_dt)
        wv_sb = big.tile([128, DK, DQ], mm_dt)
        wo_sb = big.tile([128, DQN, D], mm_dt)
        xt_all = big.tile([128, DK, S], mm_dt)

        for kt in range(DK):
            nc.sync.dma_start(xt_all[:, kt, :], m(xT[128 * kt:128 * (kt + 1), :]))

        # ===== phase A-qk: all projections, 4-moving-block stationary chains
        for dt in range(2 * DQN):
            wdt = stream.tile([128, DK, 128], mm_dt, tag="wdt", bufs=3)
            for kt in range(DK):
                nc.gpsimd.dma_start(
                    wdt[:, kt, :],
                    m(wqk[128 * kt:128 * (kt + 1), 128 * dt:128 * (dt + 1)]))
            pss = [psum.tile([128, SQ], F32, tag="qk", bufs=qk_psum_bufs,
                             name=f"pss_{dt}_{tb}") for tb in range(NSB)]
            for kt in range(DK):
                for tb in range(NSB):
                    nc.tensor.matmul(
                        pss[tb][:], m(wdt[:, kt, :]),
                        m(xt_all[:, kt, tb * SQ:(tb + 1) * SQ]),
                        start=(kt == 0), stop=(kt == DK - 1))
            is_q = dt < DQN
            hp = dt % DQN
            dest = qT if is_q else kT
            for tb in range(NSB):
                nc.scalar.activation(
                    dest[:, hp, tb * SQ:(tb + 1) * SQ], pss[tb][:],
                    mybir.ActivationFunctionType.Identity,
                    bias=bqk_sb[:, dt:dt + 1],
                    scale=0.125 if is_q else 1.0)

        def emit_v_group(blk):
            # v projection for token tiles of one superblock (token-stationary)
            for tt in range(blk * (SQ // 128), (blk + 1) * (SQ // 128)):
                psv = psum.tile([128, DQ], F32, tag="v", bufs=2,
                                name=f"psv_{tt}")
                for kt in range(DK):
                    nc.tensor.matmul(
                        psv[:], m(xt_all[:, kt, 128 * tt:128 * (tt + 1)]),
                        m(wv_sb[:, kt, :]),
                        start=(kt == 0), stop=(kt == DK - 1))
                va = v_aug[:, tt, :].rearrange("p (h c) -> p h c", h=HC)
                nc.vector.tensor_tensor(
                    va[:, :, 0:64], psv[:].rearrange("p (h c) -> p h c", h=HC),
                    bv_bc[:].rearrange("p (h c) -> p h c", h=HC),
                    op=mybir.AluOpType.add)
                nc.vector.tensor_copy(va[:, :, 64:65], ones_hc[:, :, None])

        for kt in range(DK):
            nc.gpsimd.dma_start(wv_sb[:, kt, :],
                                m(wv[128 * kt:128 * (kt + 1), :]))
        emit_v_group(0)
        for p4 in range(DQN):
            nc.gpsimd.dma_start(wo_sb[:, p4, :],
                                m(wo[128 * p4:128 * (p4 + 1), :]))

        for i in range(NSB):
            # ===== phase B: attention for superblock i =====================
            ND = SQ // 128
            NJ = ND * (i + 1)
            attnT = stream.tile([128, DQN, SQ], mm_dt, tag="attnT", bufs=2,
                                name=f"at_{i}")
            pending = [None]  # deferred (bc matmul + normalize) of prev hp
            for hp in range(DQN):
                pva = psum.tile([65, SQ], F32, tag="v", bufs=2,
                                name=f"pv_{i}_{hp}_0")
                pvb = psum.tile([65, SQ], F32, tag="v", bufs=2,
                                name=f"pv_{i}_{hp}_1")
                pvs = (pva, pvb)
                pend = None
                for j in range(NJ):
                    jj = j - ND * i
                    f0 = max(0, 128 * jj)
                    scs, prbs = [], []
                    for hh in range(2):
                        p0, p1 = 64 * hh, 64 * hh + 64
                        sc = psum.tile([128, SQ], F32, tag="qk",
                                       bufs=qk_psum_bufs,
                                       name=f"sc_{i}_{hp}_{j}_{hh}")
                        nc.tensor.matmul(
                            sc[:, f0:],
                            m(kT[p0:p1, hp, 128 * j:128 * (j + 1)]),
                            m(qT[p0:p1, hp, i * SQ + f0:(i + 1) * SQ]),
                            start=True, stop=True,
                            tile_position=(64 * hh, 0))
                        scs.append(sc)
                    if j == 1 and pending[0] is not None:
                        pending[0]()
                        pending[0] = None
                    for hh in range(2):
                        if jj >= 0:
                            nc.vector.tensor_tensor(
                                scs[hh][:, f0:f0 + 128],
                                scs[hh][:, f0:f0 + 128], tri[:],
                                op=mybir.AluOpType.add)
                        probs = stream.tile([128, SQ], mm_dt, tag="probs",
                                            bufs=probs_bufs,
                                            name=f"pr_{i}_{hp}_{j}_{hh}")
                        nc.scalar.activation(
                            probs[:, f0:], scs[hh][:, f0:],
                            mybir.ActivationFunctionType.Exp)
                        prbs.append(probs)
                    if pend is not None:
                        pprbs, pf0, pj = pend
                        for hh in range(2):
                            h = 2 * hp + hh
                            nc.tensor.matmul(
                                pvs[hh][:, pf0:],
                                m(v_aug[:, pj, 65 * h:65 * h + 65]),
                                m(pprbs[hh][:, pf0:]),
                                start=(pj == 0), stop=(pj == NJ - 1))
                    pend = (prbs, f0, j)
                pprbs, pf0, pj = pend
                for hh in range(2):
                    h = 2 * hp + hh
                    nc.tensor.matmul(
                        pvs[hh][:, pf0:],
                        m(v_aug[:, pj, 65 * h:65 * h + 65]),
                        m(pprbs[hh][:, pf0:]),
                        start=(pj == 0), stop=(pj == NJ - 1))
                recips = []
                for hh in range(2):
                    recip = stream.tile([1, SQ], F32R, tag="recip", bufs=4,
                                        name=f"rc_{i}_{hp}_{hh}")
                    nc.vector.reciprocal(recip[:], pvs[hh][64:65, :])
                    recips.append(recip)

                def make_norm(pvs=pvs, recips=recips, hp=hp, at=attnT, ii=i):
                    def emit():
                        for hh in range(2):
                            bc = psum.tile([64, SQ], F32, tag="out", bufs=2,
                                           name=f"bc_{ii}_{hp}_{hh}")
                            nc.tensor.matmul(bc[:], r(ones64r[:]),
                                             r(recips[hh][:]),
                                             start=True, stop=True)
                            bc_sb = stream.tile([64, SQ], F32, tag="bcs",
                                                bufs=2,
                                                name=f"bs_{ii}_{hp}_{hh}")
                            nc.vector.tensor_copy(bc_sb[:], bc[:])
                            if hh == 0:
                                nc.vector.tensor_tensor(
                                    at[0:64, hp, :],
                                    pvs[hh][0:64, :], bc_sb[:],
                                    op=mybir.AluOpType.mult)
                            else:
                                stage = stream.tile([64, SQ], mm_dt,
                                                    tag="stage", bufs=2,
                                                    name=f"st_{ii}_{hp}_{hh}")
                                nc.vector.tensor_tensor(
                                    stage[:], pvs[hh][0:64, :], bc_sb[:],
                                    op=mybir.AluOpType.mult)
                                nc.sync.dma_start(at[64:128, hp, :], stage[:])
                    return emit

                pending[0] = make_norm()

            if pending[0] is not None:
                pending[0]()
                pending[0] = None
            if i + 1 < NSB:
                emit_v_group(i + 1)

            # ===== phase C: output projection for superblock i ============
            for mm_ in range(SQ // 128):
                tt = i * (SQ // 128) + mm_
                pos = [psum.tile([128, NOUT], F32, tag="out", bufs=2,
                                 name=f"po_{tt}_{nb}") for nb in range(NOB)]
                for p4 in range(DQN):
                    for nb in range(NOB):
                        nc.tensor.matmul(
                            pos[nb][:],
                            m(attnT[:, p4, 128 * mm_:128 * (mm_ + 1)]),
                            m(wo_sb[:, p4, nb * NOUT:(nb + 1) * NOUT]),
                            start=(p4 == 0), stop=(p4 == DQN - 1))
                for nb in range(NOB):
                    osb = stream.tile([128, NOUT], F32, tag="osb", bufs=3,
                                      name=f"ob_{tt}_{nb}")
                    nc.vector.tensor_copy(osb[:], pos[nb][:])
                    nc.sync.dma_start(
                        out[128 * tt:128 * (tt + 1),
                            nb * NOUT:(nb + 1) * NOUT], osb[:])

    nc.compile()
    return nc

B, S, D, H = 4, 2048, 1024, 16
N_CORES = 8

_CACHED = {}


def _make_core_inputs(x, Wq, bq, Wk, bk, Wv, bv, Wo):
    DQ = D // 2

    def cast(a):
        return np.ascontiguousarray(a).astype(ml_dtypes.bfloat16)

    xTs = [cast(x[b].T) for b in range(B)]
    in_maps = []
    for c in range(N_CORES):
        b, hf = c // 2, c % 2
        sl = slice(hf * DQ, (hf + 1) * DQ)
        in_maps.append({
            "xT": xTs[b],
            "wqk": cast(np.concatenate([Wq[:, sl], Wk[:, sl]], axis=1)),
            "wv": cast(Wv[:, sl]),
            "wo": cast(Wo[sl, :]),
            "bqk": np.ascontiguousarray(
                np.concatenate([0.125 * bq[sl], bk[sl]])).astype(np.float32),
            "bv": np.ascontiguousarray(bv[sl]).astype(np.float32),
        })
    return in_maps


def kernel(x, Wq, bq, Wk, bk, Wv, bv, Wo, bo):
    import tempfile
    from concourse import bass_utils

    x = np.asarray(x, dtype=np.float32)
    Wq = np.asarray(Wq, dtype=np.float32)
    bq = np.asarray(bq, dtype=np.float32)
    Wk = np.asarray(Wk, dtype=np.float32)
    bk = np.asarray(bk, dtype=np.float32)
    Wv = np.asarray(Wv, dtype=np.float32)
    bv = np.asarray(bv, dtype=np.float32)
    Wo = np.asarray(Wo, dtype=np.float32)
    bo = np.asarray(bo, dtype=np.float32)

    if "nc" not in _CACHED:
        _CACHED["nc"] = build_core_program(S=S, D=D, HC=H // 2)
    nc = _CACHED["nc"]

    in_maps = _make_core_inputs(x, Wq, bq, Wk, bk, Wv, bv, Wo)
    res = bass_utils.run_bass_kernel_spmd(
        nc, in_maps, core_ids=list(range(N_CORES)),
        tmpdir=tempfile.mkdtemp(prefix="bass_attn_"))

    out = np.empty((B, S, D), dtype=np.float32)
    for b in range(B):
        out[b] = res.results[2 * b]["out"] + res.results[2 * b + 1]["out"] + bo
    return out



# revision 8
# speedup vs baseline: 1.2709x; 1.2709x over previous
"""Causal self-attention (B=4, S=2048, D=1024, H=16) on 8 TRN2 NeuronCores.

Sharding (tensor-parallel on heads + data-parallel on batch):
  core c -> batch c//2, head-half c%2 (8 of 16 heads).
  Wq/Wk/Wv column-split, Wo row-split; the two partial outputs per batch are
  summed on the host (+ bo), which is the row-parallel unshard.

Per-core Bass/Tile program (matmul operands bf16, psum/softmax fp32):
  phase A: qT/kT feature-major projections into 2-bank psum tiles; q is
           written as two zero-padded copies (qT0 rows 64:128 = 0, qT1 rows
           0:64 = 0) so phase-B scores matmuls use full-width [128,128] kT
           stationaries (FWL) with K=128 zero-padded moving operands.
           v token-major with 64 ones-columns per head (cols 64:128), emitted
           per-superblock interleaved with attention.
  phase B: per head-pair / 512-query superblock / 128-key tile:
           scoresT for both heads into one [128,1024] psum tile (kT full
           stationary, qT0/qT1 moving), additive triangular mask on the
           diagonal boundary subtiles, ONE merged exp per key tile, PV with
           full [128,128] v_aug stationaries whose ones-columns replicate
           sumexp into psum rows 64:128 (free broadcast); normalization is
           reciprocal_approx_fast + multiply, software-pipelined one key tile
           behind scores to hide the exp latency.
  phase C: out_partial = attnT.T @ Wo_rows (stationary reused across the two
           output column blocks).
"""

from contextlib import ExitStack

import numpy as np
import ml_dtypes

import concourse.bass as bass
import concourse.bacc as bacc
import concourse.tile as tile
import concourse.mybir as mybir

F32 = mybir.dt.float32
F32R = mybir.dt.float32r
BF16 = mybir.dt.bfloat16
NEG = -30000.0  # additive mask; must stay finite-representable in bf16 paths


def r(ap):
    return ap.bitcast(F32R)


def build_core_program(S=2048, D=1024, HC=8, DH=64, SQ=512, mm_dt=BF16,
                       qk_psum_bufs=2, probs_bufs=4):
    """Build the per-core Bass program (SPMD: same program, different data).
    mm_dt: dtype of matmul operands (BF16 or F32R). When BF16, the host must
    pass xT/wqk/wv/wo as bfloat16 arrays."""
    DQ = HC * DH              # head-slice width (512)
    DK = D // 128             # contraction tiles for projections (8)
    DQN = DQ // 128           # head-pair tiles (4)
    NSB = S // SQ             # query superblocks (4)
    NTT = S // 128            # token tiles (16)
    NOUT = min(512, D)        # output-proj free width
    NOB = D // NOUT           # output-proj col blocks (2)
    assert DQ % 128 == 0 and S % SQ == 0 and SQ % 128 == 0 and D % 128 == 0
    assert (S // SQ) % 2 == 0

    bf = mm_dt == BF16
    in_dt = BF16 if bf else F32

    def m(ap):
        # bitcast for f32->f32r reinterpretation; no-op for bf16 tiles
        return ap if bf else ap.bitcast(F32R)

    nc = bacc.Bacc("TRN2", target_bir_lowering=False, debug=False)

    xT = nc.dram_tensor("xT", [D, S], in_dt, kind="ExternalInput").ap()
    wqk = nc.dram_tensor("wqk", [D, 2 * DQ], in_dt, kind="ExternalInput").ap()
    wv = nc.dram_tensor("wv", [D, DQ], in_dt, kind="ExternalInput").ap()
    wo = nc.dram_tensor("wo", [DQ, D], in_dt, kind="ExternalInput").ap()
    bqk = nc.dram_tensor("bqk", [2 * DQ], F32, kind="ExternalInput").ap()
    bv = nc.dram_tensor("bv", [DQ], F32, kind="ExternalInput").ap()
    out = nc.dram_tensor("out", [S, D], F32, kind="ExternalOutput").ap()

    with tile.TileContext(nc) as tc, ExitStack() as ctx:
        ctx.enter_context(nc.allow_low_precision(
            reason="low-precision matmul operands; accumulation stays fp32"))
        const = ctx.enter_context(tc.tile_pool(name="const", bufs=1))
        big = ctx.enter_context(tc.tile_pool(name="big", bufs=1))
        stream = ctx.enter_context(tc.tile_pool(name="stream", bufs=1))
        psum = ctx.enter_context(tc.tile_pool(name="psum", bufs=1, space="PSUM"))

        # ---- constants ----
        # triangular mask [128,128]: 0 where p <= f else NEG (boundary subtile)
        tri = const.tile([128, 128], F32)
        nc.vector.memset(tri[:], 0.0)
        nc.gpsimd.affine_select(
            out=tri[:], in_=tri[:], compare_op=mybir.AluOpType.is_ge,
            fill=NEG, base=0, channel_multiplier=-1, pattern=[[1, 128]],
        )
        ones128f = const.tile([1, 128], F32)
        nc.vector.memset(ones128f[:], 1.0)
        ones128r = const.tile([1, 128], F32R)
        nc.vector.tensor_copy(ones128r[:], ones128f[:])

        # biases: bqk as [128, 2*DQN] (column t = dout tile t), bv broadcast
        bqk_sb = const.tile([128, 2 * DQN], F32)
        nc.sync.dma_start(bqk_sb[:], bqk.rearrange("(t p) -> p t", p=128))
        bv_rowf = const.tile([1, DQ], F32)
        nc.sync.dma_start(bv_rowf[:], bv.rearrange("(a d) -> a d", a=1))
        bv_row = const.tile([1, DQ], F32R)
        nc.vector.tensor_copy(bv_row[:], bv_rowf[:])
        bv_bc = const.tile([128, DQ], F32)
        bv_ps = psum.tile([128, DQ], F32, tag="v", bufs=2)
        nc.tensor.matmul(bv_ps[:], r(ones128r[:]), r(bv_row[:]),
                         start=True, stop=True)
        nc.scalar.copy(bv_bc[:], bv_ps[:])

        # ---- big resident tensors ----
        kT = big.tile([128, DQN, S], mm_dt)     # [pair 2x64 rows, tokens]
        qT0 = big.tile([128, DQN, S], mm_dt)    # head0 q in rows 0:64, rest 0
        qT1 = big.tile([128, DQN, S], mm_dt)    # head1 q in rows 64:128, rest 0
        # v_aug layout [p, tt, hh, hp, 128]: ones in cols 0:64, v feats in
        # cols 64:128 for every head.  During PV the ones columns replicate
        # sumexp into psum rows 0:64 (where reciprocal_approx_fast works;
        # it breaks at non-zero base partitions) and attn lands at 64:128.
        v_aug = big.tile([128, NTT, 2, DQN, 128], mm_dt)
        wv_sb = big.tile([128, DK, DQ], mm_dt)
        wo_sb = big.tile([128, DQN, D], mm_dt)
        xt_all = big.tile([128, DK, S], mm_dt)

        nc.vector.memset(qT0[64:128, :, :], 0.0)
        nc.vector.memset(qT1[0:64, :, :], 0.0)
        nc.vector.memset(v_aug[:, :, :, :, 0:DH], 1.0)

        for kt in range(DK):
            nc.sync.dma_start(xt_all[:, kt, :], m(xT[128 * kt:128 * (kt + 1), :]))

        # ===== phase A-qk: all projections, 2x2-bank stationary chains
        for dt in range(2 * DQN):
            wdt = stream.tile([128, DK, 128], mm_dt, tag="wdt", bufs=3)
            for kt in range(DK):
                nc.gpsimd.dma_start(
                    wdt[:, kt, :],
                    m(wqk[128 * kt:128 * (kt + 1), 128 * dt:128 * (dt + 1)]))
            pss = [psum.tile([128, 2 * SQ], F32, tag="qk", bufs=qk_psum_bufs,
                             name=f"pss_{dt}_{pr}") for pr in range(NSB // 2)]
            for kt in range(DK):
                for tb in range(NSB):
                    nc.tensor.matmul(
                        pss[tb // 2][:, (tb % 2) * SQ:(tb % 2 + 1) * SQ],
                        m(wdt[:, kt, :]),
                        m(xt_all[:, kt, tb * SQ:(tb + 1) * SQ]),
                        start=(kt == 0), stop=(kt == DK - 1))
            is_q = dt < DQN
            hp = dt % DQN
            for pr in range(NSB // 2):
                cols = slice(2 * SQ * pr, 2 * SQ * (pr + 1))
                if is_q:
                    nc.scalar.activation(
                        qT0[0:64, hp, cols], pss[pr][0:64, :],
                        mybir.ActivationFunctionType.Identity,
                        bias=bqk_sb[0:64, dt:dt + 1], scale=0.125)
                    nc.scalar.activation(
                        qT1[64:128, hp, cols], pss[pr][64:128, :],
                        mybir.ActivationFunctionType.Identity,
                        bias=bqk_sb[64:128, dt:dt + 1], scale=0.125)
                else:
                    nc.scalar.activation(
                        kT[:, hp, cols], pss[pr][:],
                        mybir.ActivationFunctionType.Identity,
                        bias=bqk_sb[:, dt:dt + 1], scale=1.0)

        def emit_v_group(blk):
            # v projection for token tiles of one superblock (token-stationary)
            for tt in range(blk * (SQ // 128), (blk + 1) * (SQ // 128)):
                psv = psum.tile([128, DQ], F32, tag="v", bufs=2,
                                name=f"psv_{tt}")
                for kt in range(DK):
                    nc.tensor.matmul(
                        psv[:], m(xt_all[:, kt, 128 * tt:128 * (tt + 1)]),
                        m(wv_sb[:, kt, :]),
                        start=(kt == 0), stop=(kt == DK - 1))
                # psv col = h*64 + c with h = 2*hp + hh -> [p, hh, hp, c]
                psv5 = psv[:].rearrange("p (q hh c) -> p hh q c", hh=2, c=DH)
                bv5 = bv_bc[:].rearrange("p (q hh c) -> p hh q c", hh=2, c=DH)
                nc.vector.tensor_tensor(
                    v_aug[:, tt, :, :, DH:128], psv5[:], bv5[:],
                    op=mybir.AluOpType.add)

        for kt in range(DK):
            nc.gpsimd.dma_start(wv_sb[:, kt, :],
                                m(wv[128 * kt:128 * (kt + 1), :]))
        emit_v_group(0)
        for p4 in range(DQN):
            nc.gpsimd.dma_start(wo_sb[:, p4, :],
                                m(wo[128 * p4:128 * (p4 + 1), :]))

        for i in range(NSB):
            # ===== phase B: attention for superblock i =====================
            ND = SQ // 128
            NJ = ND * (i + 1)
            attnT = stream.tile([128, DQN, SQ], mm_dt, tag="attnT", bufs=2,
                                name=f"at_{i}")
            pending = [None]  # deferred normalize of prev hp
            for hp in range(DQN):
                pva = psum.tile([128, SQ], F32, tag="v", bufs=2,
                                name=f"pv_{i}_{hp}_0")
                pvb = psum.tile([128, SQ], F32, tag="v", bufs=2,
                                name=f"pv_{i}_{hp}_1")
                pvs = (pva, pvb)
                pend = None
                for j in range(NJ):
                    jj = j - ND * i
                    f0 = max(0, 128 * jj)
                    sc = psum.tile([128, 2 * SQ], F32, tag="qk",
                                   bufs=qk_psum_bufs, name=f"sc_{i}_{hp}_{j}")
                    for hh, qsrc in ((0, qT0), (1, qT1)):
                        nc.tensor.matmul(
                            sc[:, SQ * hh + f0:SQ * (hh + 1)],
                            m(kT[:, hp, 128 * j:128 * (j + 1)]),
                            m(qsrc[:, hp, i * SQ + f0:(i + 1) * SQ]),
                            start=True, stop=True)
                    if j == 1 and pending[0] is not None:
                        pending[0]()
                        pending[0] = None
                    if jj >= 0:
                        for hh in range(2):
                            nc.vector.tensor_tensor(
                                sc[:, SQ * hh + f0:SQ * hh + f0 + 128],
                                sc[:, SQ * hh + f0:SQ * hh + f0 + 128], tri[:],
                                op=mybir.AluOpType.add)
                    probs = stream.tile([128, 2 * SQ], mm_dt, tag="probs",
                                        bufs=probs_bufs,
                                        name=f"pr_{i}_{hp}_{j}")
                    if f0 == 0:
                        nc.scalar.activation(
                            probs[:], sc[:],
                            mybir.ActivationFunctionType.Exp)
                    else:
                        nc.scalar.activation(
                            probs[:].rearrange("p (h c) -> p h c", h=2)[:, :, f0:],
                            sc[:].rearrange("p (h c) -> p h c", h=2)[:, :, f0:],
                            mybir.ActivationFunctionType.Exp)
                    if pend is not None:
                        pprbs, pf0, pj = pend
                        for hh in range(2):
                            nc.tensor.matmul(
                                pvs[hh][:, pf0:],
                                m(v_aug[:, pj, hh, hp, :]),
                                m(pprbs[:, SQ * hh + pf0:SQ * (hh + 1)]),
                                start=(pj == 0), stop=(pj == NJ - 1))
                    pend = (probs, f0, j)
                pprbs, pf0, pj = pend
                for hh in range(2):
                    nc.tensor.matmul(
                        pvs[hh][:, pf0:],
                        m(v_aug[:, pj, hh, hp, :]),
                        m(pprbs[:, SQ * hh + pf0:SQ * (hh + 1)]),
                        start=(pj == 0), stop=(pj == NJ - 1))

                def make_norm(pvs=pvs, hp=hp, at=attnT, ii=i):
                    def emit():
                        # sumexp rows 0:64 (recip works only at base
                        # partition 0), attn rows 64:128.  Reciprocal is
                        # DMA-shifted up (DVE lanes are partition-locked);
                        # hh=1 attn writes attnT rows 64:128 directly, hh=0
                        # goes through a staging tile + partition-shift DMA.
                        for hh in range(2):
                            rec = stream.tile([128, SQ], F32, tag="rec",
                                              bufs=4, name=f"rc_{ii}_{hp}_{hh}")
                            nc.vector.reciprocal_approx_fast(
                                rec[0:64, :], pvs[hh][0:64, :])
                            nc.sync.dma_start(rec[64:128, :], rec[0:64, :])
                            if hh == 1:
                                nc.vector.tensor_tensor(
                                    at[64:128, hp, :],
                                    pvs[hh][64:128, :], rec[64:128, :],
                                    op=mybir.AluOpType.mult)
                            else:
                                stage = stream.tile([128, SQ], mm_dt,
                                                    tag="stage", bufs=2,
                                                    name=f"st_{ii}_{hp}")
                                nc.vector.tensor_tensor(
                                    stage[64:128, :],
                                    pvs[hh][64:128, :], rec[64:128, :],
                                    op=mybir.AluOpType.mult)
                                nc.sync.dma_start(at[0:64, hp, :],
                                                  stage[64:128, :])
                    return emit

                pending[0] = make_norm()

            if pending[0] is not None:
                pending[0]()
                pending[0] = None
            if i + 1 < NSB:
                emit_v_group(i + 1)

            # ===== phase C: output projection for superblock i ============
            for mm_ in range(SQ // 128):
                tt = i * (SQ // 128) + mm_
                pos = [psum.tile([128, NOUT], F32, tag="out", bufs=2,
                                 name=f"po_{tt}_{nb}") for nb in range(NOB)]
                for p4 in range(DQN):
                    for nb in range(NOB):
                        nc.tensor.matmul(
                            pos[nb][:],
                            m(attnT[:, p4, 128 * mm_:128 * (mm_ + 1)]),
                            m(wo_sb[:, p4, nb * NOUT:(nb + 1) * NOUT]),
                            start=(p4 == 0), stop=(p4 == DQN - 1))
                for nb in range(NOB):
                    osb = stream.tile([128, NOUT], F32, tag="osb", bufs=3,
                                      name=f"ob_{tt}_{nb}")
                    nc.vector.tensor_copy(osb[:], pos[nb][:])
                    nc.sync.dma_start(
                        out[128 * tt:128 * (tt + 1),
                            nb * NOUT:(nb + 1) * NOUT], osb[:])

    nc.compile()
    return nc

B, S, D, H = 4, 2048, 1024, 16
N_CORES = 8

_CACHED = {}


def _make_core_inputs(x, Wq, bq, Wk, bk, Wv, bv, Wo):
    DQ = D // 2

    def cast(a):
        return np.ascontiguousarray(a).astype(ml_dtypes.bfloat16)

    xTs = [cast(x[b].T) for b in range(B)]
    in_maps = []
    for c in range(N_CORES):
        b, hf = c // 2, c % 2
        sl = slice(hf * DQ, (hf + 1) * DQ)
        in_maps.append({
            "xT": xTs[b],
            "wqk": cast(np.concatenate([Wq[:, sl], Wk[:, sl]], axis=1)),
            "wv": cast(Wv[:, sl]),
            "wo": cast(Wo[sl, :]),
            "bqk": np.ascontiguousarray(
                np.concatenate([0.125 * bq[sl], bk[sl]])).astype(np.float32),
            "bv": np.ascontiguousarray(bv[sl]).astype(np.float32),
        })
    return in_maps


def kernel(x, Wq, bq, Wk, bk, Wv, bv, Wo, bo):
    import tempfile
    from concourse import bass_utils

    x = np.asarray(x, dtype=np.float32)
    Wq = np.asarray(Wq, dtype=np.float32)
    bq = np.asarray(bq, dtype=np.float32)
    Wk = np.asarray(Wk, dtype=np.float32)
    bk = np.asarray(bk, dtype=np.float32)
    Wv = np.asarray(Wv, dtype=np.float32)
    bv = np.asarray(bv, dtype=np.float32)
    Wo = np.asarray(Wo, dtype=np.float32)
    bo = np.asarray(bo, dtype=np.float32)

    if "nc" not in _CACHED:
        _CACHED["nc"] = build_core_program(S=S, D=D, HC=H // 2)
    nc = _CACHED["nc"]

    in_maps = _make_core_inputs(x, Wq, bq, Wk, bk, Wv, bv, Wo)
    res = bass_utils.run_bass_kernel_spmd(
        nc, in_maps, core_ids=list(range(N_CORES)),
        tmpdir=tempfile.mkdtemp(prefix="bass_attn_"))

    out = np.empty((B, S, D), dtype=np.float32)
    for b in range(B):
        out[b] = res.results[2 * b]["out"] + res.results[2 * b + 1]["out"] + bo
    return out


# revision 13
# speedup vs baseline: 1.5212x; 1.1969x over previous
"""Causal self-attention (B=4, S=2048, D=1024, H=16) on 8 TRN2 NeuronCores.

Sharding (tensor-parallel on heads + data-parallel on batch):
  core c -> batch c//2, head-half c%2 (8 of 16 heads).
  Wq/Wk/Wv column-split, Wo row-split; the two partial outputs per batch are
  summed on the host (+ bo), which is the row-parallel unshard.

Per-core Bass/Tile program (matmul operands bf16, psum/softmax fp32):
  phase A: qT/kT feature-major projections into 2-bank psum tiles; q is
           written as two zero-padded copies (qT0 rows 64:128 = 0, qT1 rows
           0:64 = 0) so phase-B scores matmuls use full-width [128,128] kT
           stationaries (FWL) with K=128 zero-padded moving operands.
           v token-major with 64 ones-columns per head (cols 64:128), emitted
           per-superblock interleaved with attention.
  phase B: per head-pair / 512-query superblock / 128-key tile:
           scoresT for both heads into one [128,1024] psum tile (kT full
           stationary, qT0/qT1 moving), additive triangular mask on the
           diagonal boundary subtiles, ONE merged exp per key tile, PV with
           full [128,128] v_aug stationaries whose ones-columns replicate
           sumexp into psum rows 64:128 (free broadcast); normalization is
           reciprocal_approx_fast + multiply, software-pipelined one key tile
           behind scores to hide the exp latency.
  phase C: out_partial = attnT.T @ Wo_rows (stationary reused across the two
           output column blocks).
"""

from contextlib import ExitStack

import numpy as np
import ml_dtypes

import concourse.bass as bass
import concourse.bacc as bacc
import concourse.tile as tile
import concourse.mybir as mybir

F32 = mybir.dt.float32
F32R = mybir.dt.float32r
BF16 = mybir.dt.bfloat16
NEG = -30000.0  # additive mask; must stay finite-representable in bf16 paths


def r(ap):
    return ap.bitcast(F32R)


def build_core_program(S=2048, D=1024, HC=8, DH=64, SQ=512, mm_dt=BF16,
                       qk_psum_bufs=2, probs_bufs=4):
    """Build the per-core Bass program (SPMD: same program, different data).
    mm_dt: dtype of matmul operands (BF16 or F32R). When BF16, the host must
    pass xT/wqk/wv/wo as bfloat16 arrays."""
    DQ = HC * DH              # head-slice width (512)
    DK = D // 128             # contraction tiles for projections (8)
    DQN = DQ // 128           # head-pair tiles (4)
    NSB = S // SQ             # query superblocks (4)
    NTT = S // 128            # token tiles (16)
    NOUT = min(512, D)        # output-proj free width
    NOB = D // NOUT           # output-proj col blocks (2)
    assert DQ % 128 == 0 and S % SQ == 0 and SQ % 128 == 0 and D % 128 == 0
    assert (S // SQ) % 2 == 0

    bf = mm_dt == BF16
    in_dt = BF16 if bf else F32

    def m(ap):
        # bitcast for f32->f32r reinterpretation; no-op for bf16 tiles
        return ap if bf else ap.bitcast(F32R)

    nc = bacc.Bacc("TRN2", target_bir_lowering=False, debug=False)

    xT = nc.dram_tensor("xT", [D, S], in_dt, kind="ExternalInput").ap()
    wqk = nc.dram_tensor("wqk", [D, 2 * DQ], in_dt, kind="ExternalInput").ap()
    wv = nc.dram_tensor("wv", [D, DQ], in_dt, kind="ExternalInput").ap()
    wo = nc.dram_tensor("wo", [DQ, D], in_dt, kind="ExternalInput").ap()
    bqk = nc.dram_tensor("bqk", [2 * DQ], F32, kind="ExternalInput").ap()
    bv = nc.dram_tensor("bv", [DQ], F32, kind="ExternalInput").ap()
    out = nc.dram_tensor("out", [S, D], F32, kind="ExternalOutput").ap()

    with tile.TileContext(nc) as tc, ExitStack() as ctx:
        ctx.enter_context(nc.allow_low_precision(
            reason="low-precision matmul operands; accumulation stays fp32"))
        const = ctx.enter_context(tc.tile_pool(name="const", bufs=1))
        big = ctx.enter_context(tc.tile_pool(name="big", bufs=1))
        stream = ctx.enter_context(tc.tile_pool(name="stream", bufs=1))
        psum = ctx.enter_context(tc.tile_pool(name="psum", bufs=1, space="PSUM"))

        # ---- constants ----
        # causal mask [128,128]: 1 where p <= f else 0, multiplied into probs
        # AFTER exp (keeps the mask off the scores->exp critical chain)
        tri01 = const.tile([128, 128], mm_dt)
        nc.vector.memset(tri01[:], 1.0)
        nc.gpsimd.affine_select(
            out=tri01[:], in_=tri01[:], compare_op=mybir.AluOpType.is_ge,
            fill=0.0, base=0, channel_multiplier=-1, pattern=[[1, 128]],
        )
        ones128f = const.tile([1, 128], F32)
        nc.vector.memset(ones128f[:], 1.0)
        ones128r = const.tile([1, 128], F32R)
        nc.vector.tensor_copy(ones128r[:], ones128f[:])

        # biases: bqk as [128, 2*DQN] (column t = dout tile t), bv broadcast
        bqk_sb = const.tile([128, 2 * DQN], F32)
        nc.sync.dma_start(bqk_sb[:], bqk.rearrange("(t p) -> p t", p=128))
        bv_rowf = const.tile([1, DQ], F32)
        nc.sync.dma_start(bv_rowf[:], bv.rearrange("(a d) -> a d", a=1))
        bv_row = const.tile([1, DQ], F32R)
        nc.vector.tensor_copy(bv_row[:], bv_rowf[:])
        bv_bc = const.tile([128, DQ], F32)
        bv_ps = psum.tile([128, DQ], F32, tag="v", bufs=2)
        nc.tensor.matmul(bv_ps[:], r(ones128r[:]), r(bv_row[:]),
                         start=True, stop=True)
        nc.scalar.copy(bv_bc[:], bv_ps[:])

        # ---- big resident tensors ----
        kT = big.tile([128, DQN, S], mm_dt)     # [pair 2x64 rows, tokens]
        qT0 = big.tile([128, DQN, S], mm_dt)    # head0 q in rows 0:64, rest 0
        qT1 = big.tile([128, DQN, S], mm_dt)    # head1 q in rows 64:128, rest 0
        # v_aug layout [p, tt, hh, hp, 128]: ones in cols 0:64, v feats in
        # cols 64:128 for every head.  During PV the ones columns replicate
        # sumexp into psum rows 0:64 (where reciprocal_approx_fast works;
        # it breaks at non-zero base partitions) and attn lands at 64:128.
        v_aug = big.tile([128, NTT, 2, DQN, 128], mm_dt)
        wv_sb = big.tile([128, DK, DQ], mm_dt)
        wo_sb = big.tile([128, DQN, D], mm_dt)
        xt_all = big.tile([128, DK, S], mm_dt)

        nc.vector.memset(qT0[64:128, :, :], 0.0)
        nc.vector.memset(qT1[0:64, :, :], 0.0)
        nc.vector.memset(v_aug[:, :, :, :, 0:DH], 1.0)

        for kt in range(DK):
            nc.sync.dma_start(xt_all[:, kt, :], m(xT[128 * kt:128 * (kt + 1), :]))

        # ===== phase A-qk: all projections, 2x2-bank stationary chains
        for dt in range(2 * DQN):
            wdt = stream.tile([128, DK, 128], mm_dt, tag="wdt", bufs=3)
            for kt in range(DK):
                nc.gpsimd.dma_start(
                    wdt[:, kt, :],
                    m(wqk[128 * kt:128 * (kt + 1), 128 * dt:128 * (dt + 1)]))
            pss = [psum.tile([128, 2 * SQ], F32, tag="qk", bufs=qk_psum_bufs,
                             name=f"pss_{dt}_{pr}") for pr in range(NSB // 2)]
            for kt in range(DK):
                for tb in range(NSB):
                    nc.tensor.matmul(
                        pss[tb // 2][:, (tb % 2) * SQ:(tb % 2 + 1) * SQ],
                        m(wdt[:, kt, :]),
                        m(xt_all[:, kt, tb * SQ:(tb + 1) * SQ]),
                        start=(kt == 0), stop=(kt == DK - 1))
            is_q = dt < DQN
            hp = dt % DQN
            for pr in range(NSB // 2):
                cols = slice(2 * SQ * pr, 2 * SQ * (pr + 1))
                on_act = pr % 2 == 0  # alternate ACT/DVE to halve drain time
                if is_q:
                    for dest, rows in ((qT0, slice(0, 64)),
                                       (qT1, slice(64, 128))):
                        if on_act:
                            nc.scalar.activation(
                                dest[rows, hp, cols], pss[pr][rows, :],
                                mybir.ActivationFunctionType.Identity,
                                bias=bqk_sb[rows, dt:dt + 1], scale=0.125)
                        else:
                            nc.vector.tensor_scalar(
                                dest[rows, hp, cols], pss[pr][rows, :],
                                0.125, bqk_sb[rows, dt:dt + 1],
                                op0=mybir.AluOpType.mult,
                                op1=mybir.AluOpType.add)
                else:
                    if on_act:
                        nc.scalar.activation(
                            kT[:, hp, cols], pss[pr][:],
                            mybir.ActivationFunctionType.Identity,
                            bias=bqk_sb[:, dt:dt + 1], scale=1.0)
                    else:
                        nc.vector.tensor_scalar(
                            kT[:, hp, cols], pss[pr][:],
                            bqk_sb[:, dt:dt + 1], None,
                            op0=mybir.AluOpType.add)

        def emit_v_tile(tt):
            # v projection for one 128-token tile (token-stationary)
            psv = psum.tile([128, DQ], F32, tag="out", bufs=2,
                            name=f"psv_{tt}")
            for kt in range(DK):
                nc.tensor.matmul(
                    psv[:], m(xt_all[:, kt, 128 * tt:128 * (tt + 1)]),
                    m(wv_sb[:, kt, :]),
                    start=(kt == 0), stop=(kt == DK - 1))
            # psv col = h*64 + c with h = 2*hp + hh -> [p, hh, hp, c]
            psv5 = psv[:].rearrange("p (q hh c) -> p hh q c", hh=2, c=DH)
            bv5 = bv_bc[:].rearrange("p (q hh c) -> p hh q c", hh=2, c=DH)
            nc.vector.tensor_tensor(
                v_aug[:, tt, :, :, DH:128], psv5[:], bv5[:],
                op=mybir.AluOpType.add)

        attn_tiles = {}

        def emit_out_tile(tt):
            # output projection for one 128-token tile of a done superblock
            atp = attn_tiles[tt // (SQ // 128)]
            mm_ = tt % (SQ // 128)
            pos = [psum.tile([128, NOUT], F32, tag="out", bufs=2,
                             name=f"po_{tt}_{nb}") for nb in range(NOB)]
            for p4 in range(DQN):
                for nb in range(NOB):
                    nc.tensor.matmul(
                        pos[nb][:],
                        m(atp[:, p4, 128 * mm_:128 * (mm_ + 1)]),
                        m(wo_sb[:, p4, nb * NOUT:(nb + 1) * NOUT]),
                        start=(p4 == 0), stop=(p4 == DQN - 1))
            for nb in range(NOB):
                osb = stream.tile([128, NOUT], F32, tag="osb", bufs=3,
                                  name=f"ob_{tt}_{nb}")
                nc.vector.tensor_copy(osb[:], pos[nb][:])
                nc.sync.dma_start(
                    out[128 * tt:128 * (tt + 1),
                        nb * NOUT:(nb + 1) * NOUT], osb[:])

        for kt in range(DK):
            nc.gpsimd.dma_start(wv_sb[:, kt, :],
                                m(wv[128 * kt:128 * (kt + 1), :]))
        for tt in range(SQ // 128):
            emit_v_tile(tt)
        for p4 in range(DQN):
            nc.gpsimd.dma_start(wo_sb[:, p4, :],
                                m(wo[128 * p4:128 * (p4 + 1), :]))

        for i in range(NSB):
            # ===== phase B: attention for superblock i =====================
            ND = SQ // 128
            NJ = ND * (i + 1)
            attnT = stream.tile([128, DQN, SQ], mm_dt, tag="attnT", bufs=2,
                                name=f"at_{i}")
            attn_tiles[i] = attnT
            pending = [None]  # deferred normalize of prev hp
            for hp in range(DQN):
                pva = psum.tile([128, SQ], F32, tag="v", bufs=2,
                                name=f"pv_{i}_{hp}_0")
                pvb = psum.tile([128, SQ], F32, tag="v", bufs=2,
                                name=f"pv_{i}_{hp}_1")
                pvs = (pva, pvb)
                pend = None
                for j in range(NJ):
                    jj = j - ND * i
                    f0 = max(0, 128 * jj)
                    sc = psum.tile([128, 2 * SQ], F32, tag="qk",
                                   bufs=qk_psum_bufs, name=f"sc_{i}_{hp}_{j}")
                    for hh, qsrc in ((0, qT0), (1, qT1)):
                        nc.tensor.matmul(
                            sc[:, SQ * hh + f0:SQ * (hh + 1)],
                            m(kT[:, hp, 128 * j:128 * (j + 1)]),
                            m(qsrc[:, hp, i * SQ + f0:(i + 1) * SQ]),
                            start=True, stop=True)
                    if j == 1:
                        if pending[0] is not None:
                            pending[0]()
                            pending[0] = None
                        # PE filler while exp(j=0) runs: v proj for the next
                        # superblock's token tile hp
                        if i + 1 < NSB:
                            emit_v_tile(ND * (i + 1) + hp)
                    if j == 3 and i >= 1:
                        # PE filler: output projection of the previous
                        # superblock (its attnT is fully normalized by now)
                        emit_out_tile(ND * (i - 1) + hp)
                    probs = stream.tile([128, 2 * SQ], mm_dt, tag="probs",
                                        bufs=probs_bufs,
                                        name=f"pr_{i}_{hp}_{j}")
                    if f0 == 0:
                        nc.scalar.activation(
                            probs[:], sc[:],
                            mybir.ActivationFunctionType.Exp)
                    else:
                        nc.scalar.activation(
                            probs[:].rearrange("p (h c) -> p h c", h=2)[:, :, f0:],
                            sc[:].rearrange("p (h c) -> p h c", h=2)[:, :, f0:],
                            mybir.ActivationFunctionType.Exp)
                    if jj >= 0:
                        for hh in range(2):
                            nc.vector.tensor_tensor(
                                probs[:, SQ * hh + f0:SQ * hh + f0 + 128],
                                probs[:, SQ * hh + f0:SQ * hh + f0 + 128],
                                tri01[:], op=mybir.AluOpType.mult)
                    if pend is not None:
                        pprbs, pf0, pj = pend
                        for hh in range(2):
                            nc.tensor.matmul(
                                pvs[hh][:, pf0:],
                                m(v_aug[:, pj, hh, hp, :]),
                                m(pprbs[:, SQ * hh + pf0:SQ * (hh + 1)]),
                                start=(pj == 0), stop=(pj == NJ - 1))
                    pend = (probs, f0, j)
                pprbs, pf0, pj = pend
                for hh in range(2):
                    nc.tensor.matmul(
                        pvs[hh][:, pf0:],
                        m(v_aug[:, pj, hh, hp, :]),
                        m(pprbs[:, SQ * hh + pf0:SQ * (hh + 1)]),
                        start=(pj == 0), stop=(pj == NJ - 1))

                def make_norm(pvs=pvs, hp=hp, at=attnT, ii=i):
                    def emit():
                        # sumexp rows 0:64 (recip works only at base
                        # partition 0), attn rows 64:128.  Reciprocal is
                        # DMA-shifted up (DVE lanes are partition-locked);
                        # hh=1 attn writes attnT rows 64:128 directly, hh=0
                        # goes through a staging tile + partition-shift DMA.
                        for hh in range(2):
                            rec = stream.tile([128, SQ], F32, tag="rec",
                                              bufs=4, name=f"rc_{ii}_{hp}_{hh}")
                            nc.vector.reciprocal_approx_fast(
                                rec[0:64, :], pvs[hh][0:64, :])
                            nc.sync.dma_start(rec[64:128, :], rec[0:64, :])
                            if hh == 1:
                                nc.vector.tensor_tensor(
                                    at[64:128, hp, :],
                                    pvs[hh][64:128, :], rec[64:128, :],
                                    op=mybir.AluOpType.mult)
                            else:
                                stage = stream.tile([128, SQ], mm_dt,
                                                    tag="stage", bufs=2,
                                                    name=f"st_{ii}_{hp}")
                                nc.vector.tensor_tensor(
                                    stage[64:128, :],
                                    pvs[hh][64:128, :], rec[64:128, :],
                                    op=mybir.AluOpType.mult)
                                nc.sync.dma_start(at[0:64, hp, :],
                                                  stage[64:128, :])
                    return emit

                pending[0] = make_norm()

            if pending[0] is not None:
                pending[0]()
                pending[0] = None

        # ===== phase C tail: output projection of the last superblock ======
        for mm_ in range(SQ // 128):
            emit_out_tile((NSB - 1) * (SQ // 128) + mm_)

    nc.compile()
    return nc

B, S, D, H = 4, 2048, 1024, 16
N_CORES = 8

_CACHED = {}


def _make_core_inputs(x, Wq, bq, Wk, bk, Wv, bv, Wo):
    DQ = D // 2

    def cast(a):
        return np.ascontiguousarray(a).astype(ml_dtypes.bfloat16)

    xTs = [cast(x[b].T) for b in range(B)]
    in_maps = []
    for c in range(N_CORES):
        b, hf = c // 2, c % 2
        sl = slice(hf * DQ, (hf + 1) * DQ)
        in_maps.append({
            "xT": xTs[b],
            "wqk": cast(np.concatenate([Wq[:, sl], Wk[:, sl]], axis=1)),
            "wv": cast(Wv[:, sl]),
            "wo": cast(Wo[sl, :]),
            "bqk": np.ascontiguousarray(
                np.concatenate([0.125 * bq[sl], bk[sl]])).astype(np.float32),
            "bv": np.ascontiguousarray(bv[sl]).astype(np.float32),
        })
    return in_maps


def kernel(x, Wq, bq, Wk, bk, Wv, bv, Wo, bo):
    import tempfile
    from concourse import bass_utils

    x = np.asarray(x, dtype=np.float32)
    Wq = np.asarray(Wq, dtype=np.float32)
    bq = np.asarray(bq, dtype=np.float32)
    Wk = np.asarray(Wk, dtype=np.float32)
    bk = np.asarray(bk, dtype=np.float32)
    Wv = np.asarray(Wv, dtype=np.float32)
    bv = np.asarray(bv, dtype=np.float32)
    Wo = np.asarray(Wo, dtype=np.float32)
    bo = np.asarray(bo, dtype=np.float32)

    if "nc" not in _CACHED:
        _CACHED["nc"] = build_core_program(S=S, D=D, HC=H // 2)
    nc = _CACHED["nc"]

    in_maps = _make_core_inputs(x, Wq, bq, Wk, bk, Wv, bv, Wo)
    res = bass_utils.run_bass_kernel_spmd(
        nc, in_maps, core_ids=list(range(N_CORES)),
        tmpdir=tempfile.mkdtemp(prefix="bass_attn_"))

    out = np.empty((B, S, D), dtype=np.float32)
    for b in range(B):
        out[b] = res.results[2 * b]["out"] + res.results[2 * b + 1]["out"] + bo
    return out
